# revision 22
# baseline (speedup 1.0000x reference)
"""Trainium2 Bass kernel: multi-head self-attention block (dense transformer).

Reference computation (fp32):
    qkv = x @ w_qkv + b_qkv                  # x [b, n, dim], w_qkv [dim, 3*dim]
    q, k, v = split(qkv); heads = 16, dh = 64
    dots = (q @ k^T) * dim**-0.5  (per head)
    attn = softmax(dots, axis=-1)
    out  = (attn @ v) @ w_out + b_out        # [b, n, dim]

Sharding (8 cores): data-parallel over batch (b=2) x tensor-parallel over
head-groups (4 groups of 4 heads).  core c -> batch c//4, head-group c%4.
Each core computes q/k/v for its 4 heads only, runs attention, and multiplies
by its 256-row slice of w_out, producing a partial [n, dim] output.  The host
sums the 4 partials per batch (the "all-reduce") and adds b_out.

Device layout choices (per core):
  - host supplies x_b^T (feature-major) so no on-device transpose is needed;
    an extra ones-row k-tile folds b_qkv into the projection matmul.
  - qT, kT are computed feature-major [256, n]; v token-major [n, 256].
  - scores are computed TRANSPOSED (S^T [j, i]) so that attn@v needs no
    transpose; the two heads of a pair run concurrently on the PE via
    row-tiling (K=64 each).
  - softmax: exp on the scalar engine (PSUM -> SBUF).  attn@v uses the
    stationary operand [v_h | ones] (M=128): one matmul yields both the
    unnormalized attention output (rows 0-63) and the denominator Z
    replicated across rows 64-127, so normalization is a plain elementwise
    multiply on DVE (no partition broadcast needed).
  - ALL matmuls use moving-dim N=256: measured fp32r throughput on TRN2 is
    ~126 ns/mm at N=256 vs ~330 ns at N=512 (2.6x per column).
  - attn@v runs one j-block behind scores/exp (software pipeline) so the PE
    always has independent work while the scalar engine computes exp.
All matmul operands live in float32r tensors (single-pass PE mode;
~1.5e-4 relative rounding vs fp32).
"""

import numpy as np

import concourse.bacc as bacc
import concourse.mybir as mybir
import concourse.tile as tile
from concourse.bass_utils import run_bass_kernel_spmd

P = 128
DIM = 1024
HEADS = 16
B = 2
N = 2048
NCORES = 8
HGROUPS = 4                     # head-groups (tensor parallel)
H_LOC = HEADS // HGROUPS        # 4 heads per core
DH = DIM // HEADS               # 64
F_LOC = H_LOC * DH              # 256 features per core (per q/k/v)
SCALE = DIM ** -0.5             # exactly 1/32

F32 = mybir.dt.float32
F32R = mybir.dt.float32r
EXP = mybir.ActivationFunctionType.Exp

IC = N // 512                   # query chunks of 512 (2 matmul halves each)
JT = N // P                     # key tiles of 128
NJB = JT // 2                   # j-blocks of 2 key tiles


def build_nc(kt: int):
    """Build the single-core program (identical on all 8 cores).

    kt: number of 128-row contraction tiles for the qkv projection
        (8 for dim=1024, 9 when a ones-row block is appended to fold biases).
    """
    nc = bacc.Bacc(trn_type="TRN2")

    xT = nc.dram_tensor("xT", (kt * P, N), F32R, kind="ExternalInput")
    w = nc.dram_tensor("w", (kt * P, 3 * F_LOC), F32R, kind="ExternalInput")
    wo = nc.dram_tensor("wo", (F_LOC, DIM), F32R, kind="ExternalInput")
    out = nc.dram_tensor("out", (N, DIM), F32, kind="ExternalOutput")

    xT_t = xT[:].rearrange("(t p) n -> p t n", p=P)        # [128, kt, N]
    w_t = w[:].rearrange("(t p) f -> p t f", p=P)          # [128, kt, 768]
    wo_t = wo[:].rearrange("(t p) e -> p t e", p=P)        # [128, 2, 1024]

    with tile.TileContext(nc) as tc:
        with (
            tc.tile_pool(name="persist", bufs=1) as persist,
            tc.tile_pool(name="const", bufs=1) as const,
        ):
            qT = persist.tile([P, 2, N], F32R, tag="qT")     # [feat, ft, tok]
            kT = persist.tile([P, 2, N], F32R, tag="kT")
            # v interleaved with ones columns: slot 2h = v_h, slot 2h+1 = 1.0
            # so that lhsT = vo[:, jt, 2h:2h+2, :] is [v_h | ones] (M=128).
            vo = persist.tile([P, JT, 2 * H_LOC, DH], F32R, tag="vo")
            outT = persist.tile([P, 2, N], F32R, tag="outT")   # [hd, ktile, tok]
            wo_sb = persist.tile([P, 2, DIM], F32R, tag="wo")
            nc.gpsimd.memset(vo.bitcast(F32)[:, :, 1::2, :], 1.0)

            # ---- Phase 1: qkv projection ------------------------------
            with (
                tc.tile_pool(name="xt", bufs=2) as xt_pool,
                tc.tile_pool(name="wsb", bufs=1) as w_pool,
                tc.tile_pool(name="ps_qk", bufs=4, space="PSUM") as ps_qk,
                tc.tile_pool(name="ps_v", bufs=2, space="PSUM") as ps_v,
            ):
                w_sb = w_pool.tile([P, kt, 3 * F_LOC], F32R, tag="w")
                for k in range(kt):
                    nc.sync.dma_start(out=w_sb[:, k, :], in_=w_t[:, k, :])
                for c in range(IC):                       # token chunks of 512
                    csl = slice(c * 512, (c + 1) * 512)
                    xt = xt_pool.tile([P, kt, 512], F32R, tag="xt")
                    for k in range(kt):
                        nc.sync.dma_start(out=xt[:, k, :], in_=xT_t[:, k, csl])
                    # q^T and k^T (feature-major), N=256 halves
                    for which, dst in ((0, qT), (1, kT)):
                        for ft in range(2):
                            f0 = which * F_LOC + ft * P
                            ps = ps_qk.tile([P, 2, 256], F32, tag="psqk")
                            for ih in range(2):
                                for k in range(kt):
                                    # one accumulation group per PSUM bank:
                                    # start zeroes the whole 2KB bank, so
                                    # only the bank's first matmul starts it
                                    nc.tensor.matmul(
                                        ps[:, ih, :],
                                        lhsT=w_sb[:, k, f0:f0 + P],
                                        rhs=xt[:, k, ih * 256:(ih + 1) * 256],
                                        start=(ih == 0 and k == 0),
                                        stop=(ih == 1 and k == kt - 1),
                                        skip_group_check=True,
                                    )
                            nc.scalar.copy(out=dst[:, ft, csl], in_=ps)
                    # v (token-major), written into the even slots of vo
                    for tt in range(4):
                        psv = ps_v.tile([P, H_LOC, DH], F32, tag="psv")
                        for k in range(kt):
                            nc.tensor.matmul(
                                psv,
                                lhsT=xt[:, k, tt * P:(tt + 1) * P],
                                rhs=w_sb[:, k, 2 * F_LOC:3 * F_LOC],
                                start=(k == 0),
                                stop=(k == kt - 1),
                            )
                        nc.scalar.copy(out=vo[:, c * 4 + tt, 0::2, :], in_=psv)
                nc.sync.dma_start(out=wo_sb, in_=wo_t)

            # ---- Phase 2: attention (+ interleaved projection) ------
            # Per j-tile: 4 score matmuls (2 heads row-paired x 2 query
            # halves), 2 exps, 4 attn@v matmuls one j-tile behind (so the
            # PE never waits on the scalar engine), plus an interleaved
            # output-projection unit as PE filler.  1-bank score tiles with
            # bufs=4 give two j-tiles of slack so score matmuls never wait
            # on exp reads.
            with (
                tc.tile_pool(name="ps_s", bufs=4, space="PSUM") as ps_s,
                tc.tile_pool(name="ps_avz", bufs=1, space="PSUM") as ps_avz,
                tc.tile_pool(name="ps_fill", bufs=2, space="PSUM") as ps_fill,
                tc.tile_pool(name="expp", bufs=6) as exp_pool,
                tc.tile_pool(name="rzp", bufs=2) as rz_pool,
                tc.tile_pool(name="osb", bufs=4) as out_pool,
            ):
                def proj_unit(it, ec, copy_engine):
                    i0 = it * P
                    esl = slice(ec * 256, (ec + 1) * 256)
                    po = ps_fill.tile([P, 256], F32, tag="fill",
                                      name=f"po_{it}_{ec}")
                    for kp in range(2):
                        nc.tensor.matmul(
                            po,
                            lhsT=outT[:, kp, i0:i0 + P],
                            rhs=wo_sb[:, kp, esl],
                            start=(kp == 0),
                            stop=(kp == 1),
                        )
                    po_sb = out_pool.tile([P, 256], F32, tag="po_sb",
                                          name=f"posb_{it}_{ec}")
                    if copy_engine is nc.scalar:
                        nc.scalar.copy(out=po_sb, in_=po)
                    else:
                        nc.vector.tensor_copy(po_sb, po)
                    nc.sync.dma_start(out=out[i0:i0 + P, esl], in_=po_sb)

                # projection work for chunk ic becomes available after its
                # normalize; interleave it one chunk behind the attention.
                proj_units = []          # pending (it, ec) pairs

                for ic in range(IC):
                    isl = slice(ic * 512, (ic + 1) * 512)
                    for pr in range(2):                   # head pair
                        avz = ps_avz.tile([P, 2, 2, 256], F32, tag="avz",
                                          name=f"avz_{ic}_{pr}")
                        pend = None

                        def flush_avz(jt, exps):
                            for ih in range(2):
                                for h2 in range(2):
                                    h = pr * 2 + h2
                                    nc.tensor.matmul(
                                        avz[:, h2, ih, :],
                                        lhsT=vo[:, jt, 2 * h:2 * h + 2, :],
                                        rhs=exps[h2][:, ih, :],
                                        start=(jt == 0 and ih == 0),
                                        stop=(jt == JT - 1 and ih == 1),
                                        skip_group_check=True,
                                    )

                        for jt in range(JT):
                            ss = [ps_s.tile([P, 2, 256], F32, tag="s",
                                            name=f"s{ic}_{pr}_{jt}_{h2}")
                                  for h2 in range(2)]
                            for ih in range(2):
                                i0 = ic * 512 + ih * 256
                                for h2 in range(2):
                                    hsl = slice(h2 * DH, (h2 + 1) * DH)
                                    nc.tensor.matmul(
                                        ss[h2][:, ih, :],
                                        lhsT=kT[hsl, pr, jt * P:(jt + 1) * P],
                                        rhs=qT[hsl, pr, i0:i0 + 256],
                                        start=(ih == 0),
                                        stop=(ih == 1),
                                        skip_group_check=True,
                                    )
                            exps = []
                            for h2 in range(2):
                                e = exp_pool.tile([P, 2, 256], F32R,
                                                  tag="exp",
                                                  name=f"e{ic}_{pr}_{jt}_{h2}")
                                nc.scalar.activation(e, ss[h2], EXP)
                                exps.append(e)
                            if pend is not None:
                                flush_avz(jt - 1, pend)
                            pend = exps
                            # PE filler: one projection unit every other tile
                            if proj_units and jt % 2 == 0:
                                proj_unit(*proj_units.pop(0), nc.vector)
                        flush_avz(JT - 1, pend)

                        for h2 in range(2):
                            osl = slice(h2 * DH, (h2 + 1) * DH)
                            rz = rz_pool.tile([DH, 2, 256], F32, tag="rz")
                            nc.vector.reciprocal(rz, avz[DH:P, h2, :, :])
                            nc.vector.tensor_mul(
                                out=outT[osl, pr, isl],
                                in0=avz[0:DH, h2, :, :],
                                in1=rz,
                            )
                    proj_units.extend(
                        (ic * 4 + it, ec) for it in range(4) for ec in range(4)
                    )
                # tail: remaining projection units (scalar-engine copies --
                # the scalar engine is idle once the last exp retires)
                for it, ec in proj_units:
                    proj_unit(it, ec, nc.scalar)
    nc.finalize()
    return nc


def _shard_inputs(x, w_qkv, b_qkv, w_out):
    """Host-side sharding: per-core input dicts (see module docstring)."""
    x = np.ascontiguousarray(x, dtype=np.float32)
    w_qkv = np.asarray(w_qkv, dtype=np.float32)
    b_qkv = np.asarray(b_qkv, dtype=np.float32)
    w_out = np.asarray(w_out, dtype=np.float32)

    has_bias = bool(np.any(b_qkv))
    kt = DIM // P + (1 if has_bias else 0)

    in_maps = []
    for c in range(NCORES):
        b = c // HGROUPS
        hg = c % HGROUPS
        fsl = slice(hg * F_LOC, (hg + 1) * F_LOC)
        # per-core weight shard [dim, 768]: q (pre-scaled), k, v columns
        w_shard = np.concatenate(
            [
                w_qkv[:, 0 * DIM:1 * DIM][:, fsl] * SCALE,
                w_qkv[:, 1 * DIM:2 * DIM][:, fsl],
                w_qkv[:, 2 * DIM:3 * DIM][:, fsl],
            ],
            axis=1,
        )
        xT_aug = np.zeros((kt * P, N), dtype=np.float32)
        xT_aug[:DIM] = x[b].T
        w_aug = np.zeros((kt * P, 3 * F_LOC), dtype=np.float32)
        w_aug[:DIM] = w_shard
        if has_bias:
            xT_aug[DIM] = 1.0
            w_aug[DIM] = np.concatenate(
                [
                    b_qkv[0 * DIM:1 * DIM][fsl] * SCALE,
                    b_qkv[1 * DIM:2 * DIM][fsl],
                    b_qkv[2 * DIM:3 * DIM][fsl],
                ]
            )
        in_maps.append(
            {
                "xT": np.ascontiguousarray(xT_aug),
                "w": np.ascontiguousarray(w_aug),
                "wo": np.ascontiguousarray(w_out[fsl, :]),
            }
        )
    return in_maps, kt


def _run(x, w_qkv, b_qkv, b_out, w_out, trace=False, **spmd_kwargs):
    in_maps, kt = _shard_inputs(x, w_qkv, b_qkv, w_out)
    nc = build_nc(kt)
    res = run_bass_kernel_spmd(
        nc, in_maps, core_ids=list(range(NCORES)), trace=trace, **spmd_kwargs
    )
    b_out = np.asarray(b_out, dtype=np.float32)
    full = np.empty((B, N, DIM), dtype=np.float32)
    for b in range(B):
        acc = res.results[b * HGROUPS]["out"].astype(np.float32)
        for hg in range(1, HGROUPS):
            acc = acc + res.results[b * HGROUPS + hg]["out"]
        full[b] = acc + b_out
    return full, res


def kernel(x, w_qkv, b_qkv, w_out, b_out):
    full, _ = _run(x, w_qkv, b_qkv, b_out, w_out, trace=False)
    return full


# revision 24
# speedup vs baseline: 1.0829x; 1.0829x over previous
"""Trainium2 Bass kernel: multi-head self-attention block (dense transformer).

Reference computation (fp32):
    qkv = x @ w_qkv + b_qkv                  # x [b, n, dim], w_qkv [dim, 3*dim]
    q, k, v = split(qkv); heads = 16, dh = 64
    dots = (q @ k^T) * dim**-0.5  (per head)
    attn = softmax(dots, axis=-1)
    out  = (attn @ v) @ w_out + b_out        # [b, n, dim]

Sharding (8 cores): data-parallel over batch (b=2) x tensor-parallel over
head-groups (4 groups of 4 heads).  core c -> batch c//4, head-group c%4.
Each core computes q/k/v for its 4 heads only, runs attention, and multiplies
by its 256-row slice of w_out, producing a partial [n, dim] output.  The host
sums the 4 partials per batch (the "all-reduce") and adds b_out.

Device layout choices (per core):
  - host supplies x_b^T (feature-major) so no on-device transpose is needed;
    an extra ones-row k-tile folds b_qkv into the projection matmul.
  - qT, kT are computed feature-major [256, n]; v token-major [n, 256].
  - scores are computed TRANSPOSED (S^T [j, i]) so that attn@v needs no
    transpose; the two heads of a pair run concurrently on the PE via
    row-tiling (K=64 each).
  - softmax: exp on the scalar engine (PSUM -> SBUF).  attn@v uses the
    stationary operand [v_h | ones] (M=128): one matmul yields both the
    unnormalized attention output (rows 0-63) and the denominator Z
    replicated across rows 64-127, so normalization is a plain elementwise
    multiply on DVE (no partition broadcast needed).
  - ALL matmuls use moving-dim N=256: measured fp32r throughput on TRN2 is
    ~126 ns/mm at N=256 vs ~330 ns at N=512 (2.6x per column).
  - attn@v runs one j-block behind scores/exp (software pipeline) so the PE
    always has independent work while the scalar engine computes exp.
All matmul operands live in float32r tensors (single-pass PE mode;
~1.5e-4 relative rounding vs fp32).
"""

import numpy as np

import concourse.bacc as bacc
import concourse.mybir as mybir
import concourse.tile as tile
from concourse.bass_utils import run_bass_kernel_spmd

P = 128
DIM = 1024
HEADS = 16
B = 2
N = 2048
NCORES = 8
HGROUPS = 4                     # head-groups (tensor parallel)
H_LOC = HEADS // HGROUPS        # 4 heads per core
DH = DIM // HEADS               # 64
F_LOC = H_LOC * DH              # 256 features per core (per q/k/v)
SCALE = DIM ** -0.5             # exactly 1/32

F32 = mybir.dt.float32
F32R = mybir.dt.float32r
EXP = mybir.ActivationFunctionType.Exp

IC = N // 512                   # query chunks of 512 (2 matmul halves each)
JT = N // P                     # key tiles of 128
NJB = JT // 2                   # j-blocks of 2 key tiles


def build_nc(kt: int):
    """Build the single-core program (identical on all 8 cores).

    kt: number of 128-row contraction tiles for the qkv projection
        (8 for dim=1024, 9 when a ones-row block is appended to fold biases).
    """
    nc = bacc.Bacc(trn_type="TRN2")

    xT = nc.dram_tensor("xT", (kt * P, N), F32R, kind="ExternalInput")
    w = nc.dram_tensor("w", (kt * P, 3 * F_LOC), F32R, kind="ExternalInput")
    wo = nc.dram_tensor("wo", (F_LOC, DIM), F32R, kind="ExternalInput")
    out = nc.dram_tensor("out", (N, DIM), F32, kind="ExternalOutput")

    xT_t = xT[:].rearrange("(t p) n -> p t n", p=P)        # [128, kt, N]
    w_t = w[:].rearrange("(t p) f -> p t f", p=P)          # [128, kt, 768]
    wo_t = wo[:].rearrange("(t p) e -> p t e", p=P)        # [128, 2, 1024]

    with tile.TileContext(nc) as tc:
        with (
            tc.tile_pool(name="persist", bufs=1) as persist,
            tc.tile_pool(name="const", bufs=1) as const,
        ):
            qT = persist.tile([P, 2, N], F32R, tag="qT")     # [feat, ft, tok]
            kT = persist.tile([P, 2, N], F32R, tag="kT")
            # v interleaved with ones columns: slot 2h = v_h, slot 2h+1 = 1.0
            # so that lhsT = vo[:, jt, 2h:2h+2, :] is [v_h | ones] (M=128).
            vo = persist.tile([P, JT, 2 * H_LOC, DH], F32R, tag="vo")
            outT = persist.tile([P, 2, N], F32R, tag="outT")   # [hd, ktile, tok]
            wo_sb = persist.tile([P, 2, DIM], F32R, tag="wo")
            nc.gpsimd.memset(vo.bitcast(F32)[:, :, 1::2, :], 1.0)

            # ---- Phase 1: qkv projection ------------------------------
            with (
                tc.tile_pool(name="xt", bufs=2) as xt_pool,
                tc.tile_pool(name="wsb", bufs=1) as w_pool,
                tc.tile_pool(name="ps_qk", bufs=4, space="PSUM") as ps_qk,
                tc.tile_pool(name="ps_v", bufs=2, space="PSUM") as ps_v,
            ):
                w_sb = w_pool.tile([P, kt, 3 * F_LOC], F32R, tag="w")
                for k in range(kt):
                    nc.sync.dma_start(out=w_sb[:, k, :], in_=w_t[:, k, :])
                for c in range(IC):                       # token chunks of 512
                    csl = slice(c * 512, (c + 1) * 512)
                    xt = xt_pool.tile([P, kt, 512], F32R, tag="xt")
                    for k in range(kt):
                        nc.sync.dma_start(out=xt[:, k, :], in_=xT_t[:, k, csl])
                    # q^T and k^T (feature-major), N=256 halves
                    for which, dst in ((0, qT), (1, kT)):
                        for ft in range(2):
                            f0 = which * F_LOC + ft * P
                            ps = ps_qk.tile([P, 2, 256], F32, tag="psqk")
                            for ih in range(2):
                                for k in range(kt):
                                    # one accumulation group per PSUM bank:
                                    # start zeroes the whole 2KB bank, so
                                    # only the bank's first matmul starts it
                                    nc.tensor.matmul(
                                        ps[:, ih, :],
                                        lhsT=w_sb[:, k, f0:f0 + P],
                                        rhs=xt[:, k, ih * 256:(ih + 1) * 256],
                                        start=(ih == 0 and k == 0),
                                        stop=(ih == 1 and k == kt - 1),
                                        skip_group_check=True,
                                    )
                            nc.scalar.copy(out=dst[:, ft, csl], in_=ps)
                    # v (token-major), written into the even slots of vo
                    for tt in range(4):
                        psv = ps_v.tile([P, H_LOC, DH], F32, tag="psv")
                        for k in range(kt):
                            nc.tensor.matmul(
                                psv,
                                lhsT=xt[:, k, tt * P:(tt + 1) * P],
                                rhs=w_sb[:, k, 2 * F_LOC:3 * F_LOC],
                                start=(k == 0),
                                stop=(k == kt - 1),
                            )
                        nc.scalar.copy(out=vo[:, c * 4 + tt, 0::2, :], in_=psv)
                nc.sync.dma_start(out=wo_sb, in_=wo_t)

            # ---- Phase 2: attention (+ interleaved projection) ------
            # N=512 moving operands (fewer, bigger matmuls -> less per-
            # instruction sync overhead).  attn@v runs one j-block behind
            # scores/exp; projection units for the previous query chunk are
            # interleaved as PE filler; the remainder drains in a tail with
            # scalar-engine copies (idle once the last exp retires).
            with (
                tc.tile_pool(name="ps_s", bufs=2, space="PSUM") as ps_s,
                tc.tile_pool(name="ps_avz", bufs=1, space="PSUM") as ps_avz,
                tc.tile_pool(name="ps_fill", bufs=2, space="PSUM") as ps_fill,
                tc.tile_pool(name="expp", bufs=4) as exp_pool,
                tc.tile_pool(name="rzp", bufs=2) as rz_pool,
                tc.tile_pool(name="osb", bufs=4) as out_pool,
            ):
                def proj_unit(it, ec, scalar_copy):
                    i0 = it * P
                    esl = slice(ec * 512, (ec + 1) * 512)
                    po = ps_fill.tile([P, 512], F32, tag="fill",
                                      name=f"po_{it}_{ec}")
                    for kp in range(2):
                        nc.tensor.matmul(
                            po,
                            lhsT=outT[:, kp, i0:i0 + P],
                            rhs=wo_sb[:, kp, esl],
                            start=(kp == 0),
                            stop=(kp == 1),
                        )
                    po_sb = out_pool.tile([P, 512], F32, tag="po_sb",
                                          name=f"posb_{it}_{ec}")
                    if scalar_copy:
                        nc.scalar.copy(out=po_sb, in_=po)
                    else:
                        nc.vector.tensor_copy(po_sb, po)
                    nc.sync.dma_start(out=out[i0:i0 + P, esl], in_=po_sb)

                proj_units = []          # pending (it, ec) pairs

                for ic in range(IC):
                    isl = slice(ic * 512, (ic + 1) * 512)
                    for pr in range(2):                   # head pair
                        # avz[h2][0:64] = attn@v head pr*2+h2 (unnormalized);
                        # avz[h2][64:128] = Z replicated (ones columns of vo)
                        avz = [
                            ps_avz.tile([P, 512], F32, tag=f"avz{h2}",
                                        name=f"avz{h2}_{ic}_{pr}")
                            for h2 in range(2)
                        ]
                        pend = None

                        def flush_avz(jb, exps):
                            for jt2 in range(2):
                                jt = jb * 2 + jt2
                                first = jb == 0 and jt2 == 0
                                last = jb == NJB - 1 and jt2 == 1
                                for h2 in range(2):
                                    h = pr * 2 + h2
                                    nc.tensor.matmul(
                                        avz[h2],
                                        lhsT=vo[:, jt, 2 * h:2 * h + 2, :],
                                        rhs=exps[h2][:, jt2, :],
                                        start=first,
                                        stop=last,
                                        skip_group_check=True,
                                    )

                        for jb in range(NJB):
                            ss = [ps_s.tile([P, 2, 512], F32, tag="s",
                                            name=f"s{ic}_{pr}_{jb}_{h2}")
                                  for h2 in range(2)]
                            for jt2 in range(2):
                                jt = jb * 2 + jt2
                                for h2 in range(2):
                                    hsl = slice(h2 * DH, (h2 + 1) * DH)
                                    nc.tensor.matmul(
                                        ss[h2][:, jt2, :],
                                        lhsT=kT[hsl, pr, jt * P:(jt + 1) * P],
                                        rhs=qT[hsl, pr, isl],
                                        start=True,
                                        stop=True,
                                        skip_group_check=True,
                                    )
                            exps = []
                            for h2 in range(2):
                                e = exp_pool.tile([P, 2, 512], F32R,
                                                  tag="exp",
                                                  name=f"e{ic}_{pr}_{jb}_{h2}")
                                nc.scalar.activation(e, ss[h2], EXP)
                                exps.append(e)
                            if pend is not None:
                                flush_avz(jb - 1, pend)
                            pend = exps
                            if proj_units and jb % 2 == 1:
                                proj_unit(*proj_units.pop(0), False)
                        flush_avz(NJB - 1, pend)

                        for h2 in range(2):
                            osl = slice(h2 * DH, (h2 + 1) * DH)
                            rz = rz_pool.tile([DH, 512], F32, tag="rz")
                            nc.vector.reciprocal(rz, avz[h2][DH:P, :])
                            nc.vector.tensor_mul(
                                out=outT[osl, pr, isl],
                                in0=avz[h2][0:DH, :],
                                in1=rz,
                            )
                    proj_units.extend(
                        (ic * 4 + it, ec) for it in range(4) for ec in range(2)
                    )
                for it, ec in proj_units:
                    proj_unit(it, ec, True)
    nc.finalize()
    return nc


def _shard_inputs(x, w_qkv, b_qkv, w_out):
    """Host-side sharding: per-core input dicts (see module docstring)."""
    x = np.ascontiguousarray(x, dtype=np.float32)
    w_qkv = np.asarray(w_qkv, dtype=np.float32)
    b_qkv = np.asarray(b_qkv, dtype=np.float32)
    w_out = np.asarray(w_out, dtype=np.float32)

    has_bias = bool(np.any(b_qkv))
    kt = DIM // P + (1 if has_bias else 0)

    in_maps = []
    for c in range(NCORES):
        b = c // HGROUPS
        hg = c % HGROUPS
        fsl = slice(hg * F_LOC, (hg + 1) * F_LOC)
        # per-core weight shard [dim, 768]: q (pre-scaled), k, v columns
        w_shard = np.concatenate(
            [
                w_qkv[:, 0 * DIM:1 * DIM][:, fsl] * SCALE,
                w_qkv[:, 1 * DIM:2 * DIM][:, fsl],
                w_qkv[:, 2 * DIM:3 * DIM][:, fsl],
            ],
            axis=1,
        )
        xT_aug = np.zeros((kt * P, N), dtype=np.float32)
        xT_aug[:DIM] = x[b].T
        w_aug = np.zeros((kt * P, 3 * F_LOC), dtype=np.float32)
        w_aug[:DIM] = w_shard
        if has_bias:
            xT_aug[DIM] = 1.0
            w_aug[DIM] = np.concatenate(
                [
                    b_qkv[0 * DIM:1 * DIM][fsl] * SCALE,
                    b_qkv[1 * DIM:2 * DIM][fsl],
                    b_qkv[2 * DIM:3 * DIM][fsl],
                ]
            )
        in_maps.append(
            {
                "xT": np.ascontiguousarray(xT_aug),
                "w": np.ascontiguousarray(w_aug),
                "wo": np.ascontiguousarray(w_out[fsl, :]),
            }
        )
    return in_maps, kt


def _run(x, w_qkv, b_qkv, b_out, w_out, trace=False, **spmd_kwargs):
    in_maps, kt = _shard_inputs(x, w_qkv, b_qkv, w_out)
    nc = build_nc(kt)
    res = run_bass_kernel_spmd(
        nc, in_maps, core_ids=list(range(NCORES)), trace=trace, **spmd_kwargs
    )
    b_out = np.asarray(b_out, dtype=np.float32)
    full = np.empty((B, N, DIM), dtype=np.float32)
    for b in range(B):
        acc = res.results[b * HGROUPS]["out"].astype(np.float32)
        for hg in range(1, HGROUPS):
            acc = acc + res.results[b * HGROUPS + hg]["out"]
        full[b] = acc + b_out
    return full, res


def kernel(x, w_qkv, b_qkv, w_out, b_out):
    full, _ = _run(x, w_qkv, b_qkv, b_out, w_out, trace=False)
    return full


# revision 25
# speedup vs baseline: 1.1503x; 1.0623x over previous
"""Trainium2 Bass kernel: multi-head self-attention block (dense transformer).

Reference computation (fp32):
    qkv = x @ w_qkv + b_qkv                  # x [b, n, dim], w_qkv [dim, 3*dim]
    q, k, v = split(qkv); heads = 16, dh = 64
    dots = (q @ k^T) * dim**-0.5  (per head)
    attn = softmax(dots, axis=-1)
    out  = (attn @ v) @ w_out + b_out        # [b, n, dim]

Sharding (8 cores): data-parallel over batch (b=2) x tensor-parallel over
head-groups (4 groups of 4 heads).  core c -> batch c//4, head-group c%4.
Each core computes q/k/v for its 4 heads only, runs attention, and multiplies
by its 256-row slice of w_out, producing a partial [n, dim] output.  The host
sums the 4 partials per batch (the "all-reduce") and adds b_out.

Device layout choices (per core):
  - host supplies x_b^T (feature-major) so no on-device transpose is needed;
    an extra ones-row k-tile folds b_qkv into the projection matmul.
  - qT, kT are computed feature-major [256, n]; v token-major [n, 256].
  - scores are computed TRANSPOSED (S^T [j, i]) so that attn@v needs no
    transpose; the two heads of a pair run concurrently on the PE via
    row-tiling (K=64 each).
  - softmax: exp on the scalar engine (PSUM -> SBUF).  attn@v uses the
    stationary operand [v_h | ones] (M=128): one matmul yields both the
    unnormalized attention output (rows 0-63) and the denominator Z
    replicated across rows 64-127, so normalization is a plain elementwise
    multiply on DVE (no partition broadcast needed).
  - ALL matmuls use moving-dim N=256: measured fp32r throughput on TRN2 is
    ~126 ns/mm at N=256 vs ~330 ns at N=512 (2.6x per column).
  - attn@v runs one j-block behind scores/exp (software pipeline) so the PE
    always has independent work while the scalar engine computes exp.
All matmul operands live in float32r tensors (single-pass PE mode;
~1.5e-4 relative rounding vs fp32).
"""

import numpy as np

import concourse.bacc as bacc
import concourse.mybir as mybir
import concourse.tile as tile
from concourse.bass_utils import run_bass_kernel_spmd

P = 128
DIM = 1024
HEADS = 16
B = 2
N = 2048
NCORES = 8
HGROUPS = 4                     # head-groups (tensor parallel)
H_LOC = HEADS // HGROUPS        # 4 heads per core
DH = DIM // HEADS               # 64
F_LOC = H_LOC * DH              # 256 features per core (per q/k/v)
SCALE = DIM ** -0.5             # exactly 1/32

F32 = mybir.dt.float32
F32R = mybir.dt.float32r
EXP = mybir.ActivationFunctionType.Exp

IC = N // 512                   # query chunks of 512 (2 matmul halves each)
JT = N // P                     # key tiles of 128
NJB = JT // 2                   # j-blocks of 2 key tiles


def build_nc(kt: int):
    """Build the single-core program (identical on all 8 cores).

    kt: number of 128-row contraction tiles for the qkv projection
        (8 for dim=1024, 9 when a ones-row block is appended to fold biases).
    """
    nc = bacc.Bacc(trn_type="TRN2")

    xT = nc.dram_tensor("xT", (kt * P, N), F32R, kind="ExternalInput")
    w = nc.dram_tensor("w", (kt * P, 3 * F_LOC), F32R, kind="ExternalInput")
    wo = nc.dram_tensor("wo", (F_LOC, DIM), F32R, kind="ExternalInput")
    out = nc.dram_tensor("out", (N, DIM), F32, kind="ExternalOutput")

    xT_t = xT[:].rearrange("(t p) n -> p t n", p=P)        # [128, kt, N]
    w_t = w[:].rearrange("(t p) f -> p t f", p=P)          # [128, kt, 768]
    wo_t = wo[:].rearrange("(t p) e -> p t e", p=P)        # [128, 2, 1024]

    with tile.TileContext(nc) as tc:
        with (
            tc.tile_pool(name="persist", bufs=1) as persist,
            tc.tile_pool(name="const", bufs=1) as const,
        ):
            qT = persist.tile([P, 2, N], F32R, tag="qT")     # [feat, ft, tok]
            kT = persist.tile([P, 2, N], F32R, tag="kT")
            # v interleaved with ones columns: slot 2h = v_h, slot 2h+1 = 1.0
            # so that lhsT = vo[:, jt, 2h:2h+2, :] is [v_h | ones] (M=128).
            vo = persist.tile([P, JT, 2 * H_LOC, DH], F32R, tag="vo")
            outT = persist.tile([P, 2, N], F32R, tag="outT")   # [hd, ktile, tok]
            wo_sb = persist.tile([P, 2, DIM], F32R, tag="wo")
            nc.gpsimd.memset(vo.bitcast(F32)[:, :, 1::2, :], 1.0)

            # ---- Phase 1: qkv projection ------------------------------
            with (
                tc.tile_pool(name="xt", bufs=2) as xt_pool,
                tc.tile_pool(name="wsb", bufs=1) as w_pool,
                tc.tile_pool(name="ps_qk", bufs=4, space="PSUM") as ps_qk,
                tc.tile_pool(name="ps_v", bufs=2, space="PSUM") as ps_v,
            ):
                w_sb = w_pool.tile([P, kt, 3 * F_LOC], F32R, tag="w")
                for k in range(kt):
                    nc.sync.dma_start(out=w_sb[:, k, :], in_=w_t[:, k, :])
                for c in range(IC):                       # token chunks of 512
                    csl = slice(c * 512, (c + 1) * 512)
                    xt = xt_pool.tile([P, kt, 512], F32R, tag="xt")
                    for k in range(kt):
                        nc.sync.dma_start(out=xt[:, k, :], in_=xT_t[:, k, csl])
                    # q^T and k^T (feature-major), N=256 halves
                    for which, dst in ((0, qT), (1, kT)):
                        for ft in range(2):
                            f0 = which * F_LOC + ft * P
                            ps = ps_qk.tile([P, 2, 256], F32, tag="psqk")
                            for ih in range(2):
                                for k in range(kt):
                                    # one accumulation group per PSUM bank:
                                    # start zeroes the whole 2KB bank, so
                                    # only the bank's first matmul starts it
                                    nc.tensor.matmul(
                                        ps[:, ih, :],
                                        lhsT=w_sb[:, k, f0:f0 + P],
                                        rhs=xt[:, k, ih * 256:(ih + 1) * 256],
                                        start=(ih == 0 and k == 0),
                                        stop=(ih == 1 and k == kt - 1),
                                        skip_group_check=True,
                                    )
                            nc.scalar.copy(out=dst[:, ft, csl], in_=ps)
                    # v (token-major), written into the even slots of vo
                    for tt in range(4):
                        psv = ps_v.tile([P, H_LOC, DH], F32, tag="psv")
                        for k in range(kt):
                            nc.tensor.matmul(
                                psv,
                                lhsT=xt[:, k, tt * P:(tt + 1) * P],
                                rhs=w_sb[:, k, 2 * F_LOC:3 * F_LOC],
                                start=(k == 0),
                                stop=(k == kt - 1),
                            )
                        nc.scalar.copy(out=vo[:, c * 4 + tt, 0::2, :], in_=psv)
                nc.sync.dma_start(out=wo_sb, in_=wo_t)

            # ---- Phase 2: attention -----------------------------------
            # N=512 moving operands; attn@v one j-block behind scores/exp.
            # avz tiles are double-buffered so the DVE normalize chain
            # (reciprocal+multiply, ~8us) never blocks the next chunk's
            # attn@v accumulation.
            with (
                tc.tile_pool(name="ps_s", bufs=2, space="PSUM") as ps_s,
                tc.tile_pool(name="ps_avz", bufs=2, space="PSUM") as ps_avz,
                tc.tile_pool(name="expp", bufs=4) as exp_pool,
                tc.tile_pool(name="rzp", bufs=2) as rz_pool,
            ):
                for ic in range(IC):
                    isl = slice(ic * 512, (ic + 1) * 512)
                    for pr in range(2):                   # head pair
                        # avz[h2][0:64] = attn@v head pr*2+h2 (unnormalized);
                        # avz[h2][64:128] = Z replicated (ones columns of vo)
                        avz = [
                            ps_avz.tile([P, 512], F32, tag=f"avz{h2}",
                                        name=f"avz{h2}_{ic}_{pr}")
                            for h2 in range(2)
                        ]
                        pend = None

                        def flush_avz(jb, exps):
                            for jt2 in range(2):
                                jt = jb * 2 + jt2
                                first = jb == 0 and jt2 == 0
                                last = jb == NJB - 1 and jt2 == 1
                                for h2 in range(2):
                                    h = pr * 2 + h2
                                    nc.tensor.matmul(
                                        avz[h2],
                                        lhsT=vo[:, jt, 2 * h:2 * h + 2, :],
                                        rhs=exps[h2][:, jt2, :],
                                        start=first,
                                        stop=last,
                                        skip_group_check=True,
                                    )

                        for jb in range(NJB):
                            ss = [ps_s.tile([P, 2, 512], F32, tag="s",
                                            name=f"s{ic}_{pr}_{jb}_{h2}")
                                  for h2 in range(2)]
                            for jt2 in range(2):
                                jt = jb * 2 + jt2
                                for h2 in range(2):
                                    hsl = slice(h2 * DH, (h2 + 1) * DH)
                                    nc.tensor.matmul(
                                        ss[h2][:, jt2, :],
                                        lhsT=kT[hsl, pr, jt * P:(jt + 1) * P],
                                        rhs=qT[hsl, pr, isl],
                                        start=True,
                                        stop=True,
                                        skip_group_check=True,
                                    )
                            exps = []
                            for h2 in range(2):
                                e = exp_pool.tile([P, 2, 512], F32R,
                                                  tag="exp",
                                                  name=f"e{ic}_{pr}_{jb}_{h2}")
                                nc.scalar.activation(e, ss[h2], EXP)
                                exps.append(e)
                            if pend is not None:
                                flush_avz(jb - 1, pend)
                            pend = exps
                        flush_avz(NJB - 1, pend)

                        for h2 in range(2):
                            osl = slice(h2 * DH, (h2 + 1) * DH)
                            rz = rz_pool.tile([DH, 512], F32, tag="rz")
                            nc.vector.reciprocal(rz, avz[h2][DH:P, :])
                            nc.vector.tensor_mul(
                                out=outT[osl, pr, isl],
                                in0=avz[h2][0:DH, :],
                                in1=rz,
                            )

            # ---- Phase 3: output projection ---------------------------
            # dense PE tail; copies split between the (now idle) scalar
            # engine and DVE so neither gates the PE
            with (
                tc.tile_pool(name="ps_o", bufs=4, space="PSUM") as ps_o,
                tc.tile_pool(name="osb", bufs=8) as out_pool,
            ):
                u = 0
                for it in range(N // P):
                    i0 = it * P
                    for ec in range(2):
                        esl = slice(ec * 512, (ec + 1) * 512)
                        po = ps_o.tile([P, 512], F32, tag="po")
                        for kp in range(2):
                            nc.tensor.matmul(
                                po,
                                lhsT=outT[:, kp, i0:i0 + P],
                                rhs=wo_sb[:, kp, esl],
                                start=(kp == 0),
                                stop=(kp == 1),
                            )
                        po_sb = out_pool.tile([P, 512], F32, tag="po_sb",
                                              name=f"posb_{it}_{ec}")
                        if u % 2 == 0:
                            nc.scalar.copy(out=po_sb, in_=po)
                        else:
                            nc.vector.tensor_copy(po_sb, po)
                        u += 1
                        nc.sync.dma_start(out=out[i0:i0 + P, esl], in_=po_sb)
    nc.finalize()
    return nc


def _shard_inputs(x, w_qkv, b_qkv, w_out):
    """Host-side sharding: per-core input dicts (see module docstring)."""
    x = np.ascontiguousarray(x, dtype=np.float32)
    w_qkv = np.asarray(w_qkv, dtype=np.float32)
    b_qkv = np.asarray(b_qkv, dtype=np.float32)
    w_out = np.asarray(w_out, dtype=np.float32)

    has_bias = bool(np.any(b_qkv))
    kt = DIM // P + (1 if has_bias else 0)

    in_maps = []
    for c in range(NCORES):
        b = c // HGROUPS
        hg = c % HGROUPS
        fsl = slice(hg * F_LOC, (hg + 1) * F_LOC)
        # per-core weight shard [dim, 768]: q (pre-scaled), k, v columns
        w_shard = np.concatenate(
            [
                w_qkv[:, 0 * DIM:1 * DIM][:, fsl] * SCALE,
                w_qkv[:, 1 * DIM:2 * DIM][:, fsl],
                w_qkv[:, 2 * DIM:3 * DIM][:, fsl],
            ],
            axis=1,
        )
        xT_aug = np.zeros((kt * P, N), dtype=np.float32)
        xT_aug[:DIM] = x[b].T
        w_aug = np.zeros((kt * P, 3 * F_LOC), dtype=np.float32)
        w_aug[:DIM] = w_shard
        if has_bias:
            xT_aug[DIM] = 1.0
            w_aug[DIM] = np.concatenate(
                [
                    b_qkv[0 * DIM:1 * DIM][fsl] * SCALE,
                    b_qkv[1 * DIM:2 * DIM][fsl],
                    b_qkv[2 * DIM:3 * DIM][fsl],
                ]
            )
        in_maps.append(
            {
                "xT": np.ascontiguousarray(xT_aug),
                "w": np.ascontiguousarray(w_aug),
                "wo": np.ascontiguousarray(w_out[fsl, :]),
            }
        )
    return in_maps, kt


def _run(x, w_qkv, b_qkv, b_out, w_out, trace=False, **spmd_kwargs):
    in_maps, kt = _shard_inputs(x, w_qkv, b_qkv, w_out)
    nc = build_nc(kt)
    res = run_bass_kernel_spmd(
        nc, in_maps, core_ids=list(range(NCORES)), trace=trace, **spmd_kwargs
    )
    b_out = np.asarray(b_out, dtype=np.float32)
    full = np.empty((B, N, DIM), dtype=np.float32)
    for b in range(B):
        acc = res.results[b * HGROUPS]["out"].astype(np.float32)
        for hg in range(1, HGROUPS):
            acc = acc + res.results[b * HGROUPS + hg]["out"]
        full[b] = acc + b_out
    return full, res


def kernel(x, w_qkv, b_qkv, w_out, b_out):
    full, _ = _run(x, w_qkv, b_qkv, b_out, w_out, trace=False)
    return full


# revision 27
# speedup vs baseline: 1.3522x; 1.1755x over previous
"""Trainium2 Bass kernel: multi-head self-attention block (dense transformer).

Reference computation (fp32):
    qkv = x @ w_qkv + b_qkv                  # x [b, n, dim], w_qkv [dim, 3*dim]
    q, k, v = split(qkv); heads = 16, dh = 64
    dots = (q @ k^T) * dim**-0.5  (per head)
    attn = softmax(dots, axis=-1)
    out  = (attn @ v) @ w_out + b_out        # [b, n, dim]

Sharding (8 cores): data-parallel over batch (b=2) x tensor-parallel over
head-groups (4 groups of 4 heads).  core c -> batch c//4, head-group c%4.
Each core computes q/k/v for its 4 heads only, runs attention, and multiplies
by its 256-row slice of w_out, producing a partial [n, dim] output.  The host
sums the 4 partials per batch (the "all-reduce") and adds b_out.

Device layout choices (per core):
  - host supplies x_b^T (feature-major) so no on-device transpose is needed;
    an extra ones-row k-tile folds b_qkv into the projection matmul.
  - qT, kT are computed feature-major [256, n]; v token-major [n, 256].
  - scores are computed TRANSPOSED (S^T [j, i]) so that attn@v needs no
    transpose; the two heads of a pair run concurrently on the PE via
    row-tiling (K=64 each).
  - softmax: exp on the scalar engine (PSUM -> SBUF).  attn@v uses the
    stationary operand [v_h | ones] (M=128): one matmul yields both the
    unnormalized attention output (rows 0-63) and the denominator Z
    replicated across rows 64-127, so normalization is a plain elementwise
    multiply on DVE (no partition broadcast needed).
  - ALL matmuls use moving-dim N=256: measured fp32r throughput on TRN2 is
    ~126 ns/mm at N=256 vs ~330 ns at N=512 (2.6x per column).
  - attn@v runs one j-block behind scores/exp (software pipeline) so the PE
    always has independent work while the scalar engine computes exp.
All matmul operands live in float32r tensors (single-pass PE mode;
~1.5e-4 relative rounding vs fp32).
"""

import numpy as np

import concourse.bacc as bacc
import concourse.mybir as mybir
import concourse.tile as tile
from concourse.bass_utils import run_bass_kernel_spmd

P = 128
DIM = 1024
HEADS = 16
B = 2
N = 2048
NCORES = 8
HGROUPS = 4                     # head-groups (tensor parallel)
H_LOC = HEADS // HGROUPS        # 4 heads per core
DH = DIM // HEADS               # 64
F_LOC = H_LOC * DH              # 256 features per core (per q/k/v)
SCALE = DIM ** -0.5             # exactly 1/32

F32 = mybir.dt.float32
F32R = mybir.dt.float32r
EXP = mybir.ActivationFunctionType.Exp

IC = N // 512                   # query chunks of 512 (2 matmul halves each)
JT = N // P                     # key tiles of 128
NJB = JT // 2                   # j-blocks of 2 key tiles


def build_nc(kt: int):
    """Build the single-core program (identical on all 8 cores).

    kt: number of 128-row contraction tiles for the qkv projection
        (8 for dim=1024, 9 when a ones-row block is appended to fold biases).
    """
    nc = bacc.Bacc(trn_type="TRN2")

    xT = nc.dram_tensor("xT", (kt * P, N), F32R, kind="ExternalInput")
    w = nc.dram_tensor("w", (kt * P, 3 * F_LOC), F32R, kind="ExternalInput")
    wo = nc.dram_tensor("wo", (F_LOC, DIM), F32R, kind="ExternalInput")
    out = nc.dram_tensor("out", (N, DIM), F32, kind="ExternalOutput")

    xT_t = xT[:].rearrange("(t p) n -> p t n", p=P)        # [128, kt, N]
    w_t = w[:].rearrange("(t p) f -> p t f", p=P)          # [128, kt, 768]
    wo_t = wo[:].rearrange("(t p) e -> p t e", p=P)        # [128, 2, 1024]

    with tile.TileContext(nc) as tc:
        with (
            tc.tile_pool(name="persist", bufs=1) as persist,
            tc.tile_pool(name="const", bufs=1) as const,
        ):
            qT = persist.tile([P, 2, N], F32R, tag="qT")     # [feat, ft, tok]
            kT = persist.tile([P, 2, N], F32R, tag="kT")
            # v interleaved with ones columns: slot 2h = v_h, slot 2h+1 = 1.0
            # so that lhsT = vo[:, jt, 2h:2h+2, :] is [v_h | ones] (M=128).
            vo = persist.tile([P, JT, 2 * H_LOC, DH], F32R, tag="vo")
            outT = persist.tile([P, 2, N], F32R, tag="outT")   # [hd, ktile, tok]
            wo_sb = persist.tile([P, 2, DIM], F32R, tag="wo")
            nc.gpsimd.memset(vo.bitcast(F32)[:, :, 1::2, :], 1.0)

            # ---- Phase 1: qkv projection ------------------------------
            with (
                tc.tile_pool(name="xt", bufs=2) as xt_pool,
                tc.tile_pool(name="wsb", bufs=1) as w_pool,
                tc.tile_pool(name="ps_qk", bufs=4, space="PSUM") as ps_qk,
                tc.tile_pool(name="ps_v", bufs=2, space="PSUM") as ps_v,
            ):
                w_sb = w_pool.tile([P, kt, 3 * F_LOC], F32R, tag="w")
                for k in range(kt):
                    nc.sync.dma_start(out=w_sb[:, k, :], in_=w_t[:, k, :])
                for c in range(IC):                       # token chunks of 512
                    csl = slice(c * 512, (c + 1) * 512)
                    xt = xt_pool.tile([P, kt, 512], F32R, tag="xt")
                    for k in range(kt):
                        nc.sync.dma_start(out=xt[:, k, :], in_=xT_t[:, k, csl])
                    # q^T and k^T (feature-major), N=256 halves
                    for which, dst in ((0, qT), (1, kT)):
                        for ft in range(2):
                            f0 = which * F_LOC + ft * P
                            ps = ps_qk.tile([P, 2, 256], F32, tag="psqk")
                            for ih in range(2):
                                for k in range(kt):
                                    # one accumulation group per PSUM bank:
                                    # start zeroes the whole 2KB bank, so
                                    # only the bank's first matmul starts it
                                    nc.tensor.matmul(
                                        ps[:, ih, :],
                                        lhsT=w_sb[:, k, f0:f0 + P],
                                        rhs=xt[:, k, ih * 256:(ih + 1) * 256],
                                        start=(ih == 0 and k == 0),
                                        stop=(ih == 1 and k == kt - 1),
                                        skip_group_check=True,
                                    )
                            nc.scalar.copy(out=dst[:, ft, csl], in_=ps)
                    # v (token-major), written into the even slots of vo
                    for tt in range(4):
                        psv = ps_v.tile([P, H_LOC, DH], F32, tag="psv")
                        for k in range(kt):
                            nc.tensor.matmul(
                                psv,
                                lhsT=xt[:, k, tt * P:(tt + 1) * P],
                                rhs=w_sb[:, k, 2 * F_LOC:3 * F_LOC],
                                start=(k == 0),
                                stop=(k == kt - 1),
                            )
                        nc.scalar.copy(out=vo[:, c * 4 + tt, 0::2, :], in_=psv)
                nc.sync.dma_start(out=wo_sb, in_=wo_t)

            # ---- Phase 2: attention -----------------------------------
            # N=512 moving operands; attn@v one j-block behind scores/exp.
            # avz tiles are double-buffered so the DVE normalize chain
            # (reciprocal+multiply, ~8us) never blocks the next chunk's
            # attn@v accumulation.
            with (
                tc.tile_pool(name="ps_s", bufs=2, space="PSUM") as ps_s,
                tc.tile_pool(name="ps_avz", bufs=2, space="PSUM") as ps_avz,
                tc.tile_pool(name="expp", bufs=4) as exp_pool,
                tc.tile_pool(name="rzp", bufs=4) as rz_pool,
            ):
                for ic in range(IC):
                    isl = slice(ic * 512, (ic + 1) * 512)
                    for pr in range(2):                   # head pair
                        # avz[h2][0:64] = attn@v head pr*2+h2 (unnormalized);
                        # avz[h2][64:128] = Z replicated (ones columns of vo)
                        avz = [
                            ps_avz.tile([P, 512], F32, tag=f"avz{h2}",
                                        name=f"avz{h2}_{ic}_{pr}")
                            for h2 in range(2)
                        ]
                        pend = None

                        def flush_avz(jb, exps):
                            for jt2 in range(2):
                                jt = jb * 2 + jt2
                                first = jb == 0 and jt2 == 0
                                last = jb == NJB - 1 and jt2 == 1
                                for h2 in range(2):
                                    h = pr * 2 + h2
                                    nc.tensor.matmul(
                                        avz[h2],
                                        lhsT=vo[:, jt, 2 * h:2 * h + 2, :],
                                        rhs=exps[h2][:, jt2, :],
                                        start=first,
                                        stop=last,
                                        skip_group_check=True,
                                    )

                        for jb in range(NJB):
                            ss = [ps_s.tile([P, 2, 512], F32, tag="s",
                                            name=f"s{ic}_{pr}_{jb}_{h2}")
                                  for h2 in range(2)]
                            for jt2 in range(2):
                                jt = jb * 2 + jt2
                                for h2 in range(2):
                                    hsl = slice(h2 * DH, (h2 + 1) * DH)
                                    nc.tensor.matmul(
                                        ss[h2][:, jt2, :],
                                        lhsT=kT[hsl, pr, jt * P:(jt + 1) * P],
                                        rhs=qT[hsl, pr, isl],
                                        start=True,
                                        stop=True,
                                        skip_group_check=True,
                                    )
                            exps = []
                            for h2 in range(2):
                                e = exp_pool.tile([P, 2, 512], F32R,
                                                  tag="exp",
                                                  name=f"e{ic}_{pr}_{jb}_{h2}")
                                nc.scalar.activation(e, ss[h2], EXP)
                                exps.append(e)
                            if pend is not None:
                                flush_avz(jb - 1, pend)
                            pend = exps
                        flush_avz(NJB - 1, pend)

                        # copy avz out of PSUM first (fast, releases the
                        # PSUM bank); the slow reciprocal then runs on the
                        # SBUF copy, off the PE/ACT critical path.
                        avzsb = []
                        for h2 in range(2):
                            t = rz_pool.tile([P, 512], F32, tag="avzsb",
                                             name=f"avzsb{h2}_{ic}_{pr}")
                            nc.vector.tensor_copy(t, avz[h2])
                            avzsb.append(t)
                        for h2 in range(2):
                            osl = slice(h2 * DH, (h2 + 1) * DH)
                            rz = rz_pool.tile([DH, 512], F32, tag="rz")
                            nc.vector.reciprocal(rz, avzsb[h2][DH:P, :])
                            nc.vector.tensor_mul(
                                out=outT[osl, pr, isl],
                                in0=avzsb[h2][0:DH, :],
                                in1=rz,
                            )

            # ---- Phase 3: output projection ---------------------------
            # dense PE tail; copies split between the (now idle) scalar
            # engine and DVE so neither gates the PE
            with (
                tc.tile_pool(name="ps_o", bufs=4, space="PSUM") as ps_o,
                tc.tile_pool(name="osb", bufs=8) as out_pool,
            ):
                u = 0
                for it in range(N // P):
                    i0 = it * P
                    for ec in range(2):
                        esl = slice(ec * 512, (ec + 1) * 512)
                        po = ps_o.tile([P, 512], F32, tag="po")
                        for kp in range(2):
                            nc.tensor.matmul(
                                po,
                                lhsT=outT[:, kp, i0:i0 + P],
                                rhs=wo_sb[:, kp, esl],
                                start=(kp == 0),
                                stop=(kp == 1),
                            )
                        po_sb = out_pool.tile([P, 512], F32, tag="po_sb",
                                              name=f"posb_{it}_{ec}")
                        if u % 2 == 0:
                            nc.scalar.copy(out=po_sb, in_=po)
                        else:
                            nc.vector.tensor_copy(po_sb, po)
                        u += 1
                        nc.sync.dma_start(out=out[i0:i0 + P, esl], in_=po_sb)
    nc.finalize()
    return nc


def _shard_inputs(x, w_qkv, b_qkv, w_out):
    """Host-side sharding: per-core input dicts (see module docstring)."""
    x = np.ascontiguousarray(x, dtype=np.float32)
    w_qkv = np.asarray(w_qkv, dtype=np.float32)
    b_qkv = np.asarray(b_qkv, dtype=np.float32)
    w_out = np.asarray(w_out, dtype=np.float32)

    has_bias = bool(np.any(b_qkv))
    kt = DIM // P + (1 if has_bias else 0)

    in_maps = []
    for c in range(NCORES):
        b = c // HGROUPS
        hg = c % HGROUPS
        fsl = slice(hg * F_LOC, (hg + 1) * F_LOC)
        # per-core weight shard [dim, 768]: q (pre-scaled), k, v columns
        w_shard = np.concatenate(
            [
                w_qkv[:, 0 * DIM:1 * DIM][:, fsl] * SCALE,
                w_qkv[:, 1 * DIM:2 * DIM][:, fsl],
                w_qkv[:, 2 * DIM:3 * DIM][:, fsl],
            ],
            axis=1,
        )
        xT_aug = np.zeros((kt * P, N), dtype=np.float32)
        xT_aug[:DIM] = x[b].T
        w_aug = np.zeros((kt * P, 3 * F_LOC), dtype=np.float32)
        w_aug[:DIM] = w_shard
        if has_bias:
            xT_aug[DIM] = 1.0
            w_aug[DIM] = np.concatenate(
                [
                    b_qkv[0 * DIM:1 * DIM][fsl] * SCALE,
                    b_qkv[1 * DIM:2 * DIM][fsl],
                    b_qkv[2 * DIM:3 * DIM][fsl],
                ]
            )
        in_maps.append(
            {
                "xT": np.ascontiguousarray(xT_aug),
                "w": np.ascontiguousarray(w_aug),
                "wo": np.ascontiguousarray(w_out[fsl, :]),
            }
        )
    return in_maps, kt


def _run(x, w_qkv, b_qkv, b_out, w_out, trace=False, **spmd_kwargs):
    in_maps, kt = _shard_inputs(x, w_qkv, b_qkv, w_out)
    nc = build_nc(kt)
    res = run_bass_kernel_spmd(
        nc, in_maps, core_ids=list(range(NCORES)), trace=trace, **spmd_kwargs
    )
    b_out = np.asarray(b_out, dtype=np.float32)
    full = np.empty((B, N, DIM), dtype=np.float32)
    for b in range(B):
        acc = res.results[b * HGROUPS]["out"].astype(np.float32)
        for hg in range(1, HGROUPS):
            acc = acc + res.results[b * HGROUPS + hg]["out"]
        full[b] = acc + b_out
    return full, res


def kernel(x, w_qkv, b_qkv, w_out, b_out):
    full, _ = _run(x, w_qkv, b_qkv, b_out, w_out, trace=False)
    return full
